# revision 3
# baseline (speedup 1.0000x reference)
"""Trainium2 Bass kernel for localized 3x3-window multi-head attention.

Problem: B=8, N=4096 (64x64 grid), DIM=512, 8 heads x 64 dim, KSIZE=3.
  qkv = x @ w_qkv; per-head localized attention over zero-padded 3x3
  spatial neighborhood; out = attn_out @ w_out + b_out.

Sharding: data-parallel over batch — core i computes batch i (8 cores).

Per-core pipeline (v3: head-major dots, token-major softmax/AV):
  MM1-qk (PE): qT/kT produced head-major (lhsT = w_qkv chunk stationary,
      rhs = host-pretransposed xT) so the 9 window shifts of K are pure
      free-dim (token) slices of a haloed SBUF tile — no partition shifts
      and no DRAM roundtrip for K.
  dots (DVE+PE): one broadcast-AP bf16 multiply per 2-head block builds all
      9 shifted q*k products; constant ones-block matmuls contract d on the
      PE, PSUM-accumulating the four blocks into one [8, 9*128] tile. Tiny
      PE transposes return dots to token-major.
  softmax (ACT/DVE, token-major): exp without max-subtraction (logits are
      provably small for this input distribution); grid-row wrap edges fixed
      by a constant mask, their exp(0)=1 mass restored into Z via a constant
      per-partition count.
  AV (DVE): V goes token-major through a zero-padded DRAM scratch; the 9
      shifted V tiles arrive as 3 strided DMAs; attn weights are expanded
      over head_dim (ACT), then 9 mul + 8 add bf16 ops.
  MM2 (PE): attn_out transposed on PE, out = lhsT.T @ w_out + b_out.
"""

import numpy as np
import ml_dtypes

bf16 = ml_dtypes.bfloat16

B, N, DIM = 8, 4096, 512
HEADS, HEAD_DIM, K9 = 8, 64, 9
GRID = 64          # 64x64 spatial grid
PAD = 65           # max |token shift| = 64+1
NT = N // 128      # 32 token tiles per core
NC4 = N // 512     # 8 512-token chunks per core
VROWS = N + 2 * PAD
HALO = 2 * PAD + 512   # haloed token window per chunk (642)

_CACHE = {}

# opt-in profiling knobs (test.py sets these; harness leaves defaults)
TRACE = False
TRACE_DIR = None
LAST_RESULTS = None


def _build():
    import concourse.bass as bass
    import concourse.mybir as mybir
    import concourse.tile as tile
    from concourse import bacc
    from concourse.bass import ts
    from concourse.masks import make_identity

    fp32 = mybir.dt.float32
    b16 = mybir.dt.bfloat16
    Copy = mybir.ActivationFunctionType.Copy

    nc = bacc.Bacc("TRN2", target_bir_lowering=False, debug=False)

    xT = nc.dram_tensor("xT", [DIM, N], b16, kind="ExternalInput")
    wq = nc.dram_tensor("wq", [DIM, 3 * DIM], b16, kind="ExternalInput")
    wo = nc.dram_tensor("wo", [DIM, DIM], b16, kind="ExternalInput")
    bbc = nc.dram_tensor("bbc", [128, DIM], fp32, kind="ExternalInput")
    # wrap mask / wrap count, [kk, h] layout
    wmask = nc.dram_tensor("wmask", [128, K9 * HEADS], fp32, kind="ExternalInput")
    nw = nc.dram_tensor("nw", [128, 1], fp32, kind="ExternalInput")
    out = nc.dram_tensor("out", [N, DIM], fp32, kind="ExternalOutput")

    with tile.TileContext(nc) as tc:
        with (
            tc.tile_pool(name="const", bufs=1) as const,
            tc.tile_pool(name="dram", bufs=1, space="DRAM") as dpool,
            tc.tile_pool(name="qkt", bufs=2) as qktpool,
            tc.tile_pool(name="vs", bufs=3) as vspool,
            tc.tile_pool(name="v9", bufs=5) as v9pool,
            tc.tile_pool(name="attn", bufs=3) as apool,
            tc.tile_pool(name="outp", bufs=3) as opool,
            tc.tile_pool(name="psqk", bufs=2, space="PSUM") as psqk,
            tc.tile_pool(name="psd", bufs=1, space="PSUM") as psd,
            tc.tile_pool(name="psdt", bufs=1, space="PSUM") as psdt,
            tc.tile_pool(name="pstp", bufs=1, space="PSUM") as pstp,
            tc.tile_pool(name="psm2", bufs=1, space="PSUM") as psm2,
        ):
            # ---- constants ----
            xT_sb = [const.tile([128, N], b16, name=f"xT{c}") for c in range(4)]
            wq_sb = [const.tile([128, 3 * DIM], b16, name=f"wq{c}") for c in range(4)]
            wo_sb = [const.tile([128, DIM], b16, name=f"wo{c}") for c in range(4)]
            for c in range(4):
                nc.sync.dma_start(out=xT_sb[c], in_=xT[ts(c, 128), :])
                nc.sync.dma_start(out=wq_sb[c], in_=wq[ts(c, 128), :])
                nc.sync.dma_start(out=wo_sb[c], in_=wo[ts(c, 128), :])
            bbc_sb = const.tile([128, DIM], fp32, name="bbc")
            nc.sync.dma_start(out=bbc_sb, in_=bbc[:, :])
            wm_sb = const.tile([128, K9 * HEADS], fp32, name="wm")
            nc.sync.dma_start(out=wm_sb, in_=wmask[:, :])
            nw_sb = const.tile([128, 1], fp32, name="nw")
            nc.sync.dma_start(out=nw_sb, in_=nw[:, :])
            ident = const.tile([128, 128], b16, name="ident")
            make_identity(nc, ident)
            zero_sb = const.tile([128, DIM], b16, name="zero")
            nc.vector.memset(zero_sb, 0.0)
            # per-block ones weights: block b maps its two 64-partition head
            # segments to psum rows 2b / 2b+1 (other columns zero, so PSUM
            # accumulation stacks the four blocks into one [8, n] tile)
            onesb = []
            for b in range(4):
                ob = const.tile([128, 8], b16, name=f"onesw{b}")
                nc.vector.memset(ob, 0.0)
                for h2 in range(2):
                    nc.vector.memset(
                        ob[64 * h2:64 * (h2 + 1), 2 * b + h2:2 * b + h2 + 1],
                        1.0)
                onesb.append(ob)

            # ---- V DRAM scratch with zero pad rows ----
            vdr = dpool.tile([VROWS, DIM], b16, name="vscratch")
            nc.sync.dma_start(out=vdr[0:PAD, :], in_=zero_sb[0:PAD, :])
            nc.sync.dma_start(out=vdr[PAD + N:VROWS, :], in_=zero_sb[0:PAD, :])

            qT_tiles = {}   # chunk -> [4 blocks] of [128, 512]
            kT_tiles = {}   # chunk -> [4 blocks] of [128, HALO]
            v9_tiles = {}   # tile -> [128, 3, 3, DIM]

            def mm1(C):
                """Chunk C (512 tokens): qT,kT head-major; V token-major."""
                qTb = [qktpool.tile([128, 512], b16, tag=f"qT{b}",
                                    name=f"qT{b}_{C}") for b in range(4)]
                kTb = [qktpool.tile([128, HALO], b16, tag=f"kT{b}",
                                    name=f"kT{b}_{C}") for b in range(4)]
                qT_tiles[C] = qTb
                kT_tiles[C] = kTb
                for b in range(4):
                    for which in range(2):  # 0 = q, 1 = k
                        ps = psqk.tile([128, 512], fp32, tag="qk")
                        mcol = which * DIM + b * 128
                        for c in range(4):
                            nc.tensor.matmul(
                                ps, lhsT=wq_sb[c][:, mcol:mcol + 128],
                                rhs=xT_sb[c][:, ts(C, 512)],
                                start=(c == 0), stop=(c == 3))
                        if which == 0:
                            # fold attention scale into qT
                            nc.scalar.activation(qTb[b], ps, Copy, scale=0.125)
                        else:
                            nc.scalar.activation(kTb[b][:, PAD:PAD + 512], ps,
                                                 Copy)
                            if C > 0:
                                # my first 65 tokens are C-1's right halo
                                nc.scalar.activation(
                                    kT_tiles[C - 1][b][:, PAD + 512:HALO],
                                    ps[:, 0:PAD], Copy)
                                # C-1's last 65 tokens are my left halo
                                nc.vector.tensor_copy(
                                    kTb[b][:, 0:PAD],
                                    kT_tiles[C - 1][b][:, 512:512 + PAD])
                            else:
                                nc.vector.memset(kTb[b][:, 0:PAD], 0.0)
                            if C == NC4 - 1:
                                nc.vector.memset(kTb[b][:, PAD + 512:HALO], 0.0)
                # V token-major, per 128-token tile
                for tt in range(4):
                    t = 4 * C + tt
                    psv_t = psqk.tile([128, DIM], fp32, tag="qk")
                    for c in range(4):
                        nc.tensor.matmul(
                            psv_t, lhsT=xT_sb[c][:, ts(t, 128)],
                            rhs=wq_sb[c][:, 2 * DIM:3 * DIM],
                            start=(c == 0), stop=(c == 3))
                    vt = vspool.tile([128, DIM], b16, tag="vst")
                    nc.scalar.activation(vt, psv_t, Copy)
                    nc.gpsimd.dma_start(
                        out=vdr[PAD + t * 128: PAD + (t + 1) * 128, :], in_=vt)

            def prefetch_v(t):
                v9t = v9pool.tile([128, 3, 3, DIM], b16, tag="v9")
                for di in range(3):
                    base = (t * 128 + 64 * di) * DIM
                    src = bass.AP(tensor=vdr.tensor, offset=vdr.offset + base,
                                  ap=[[DIM, 128], [DIM, 3], [1, DIM]])
                    nc.sync.dma_start(out=v9t[:, di, :, :], in_=src)
                v9_tiles[t] = v9t

            def attn(t):
                C, tt = t // 4, t % 4
                qTb, kTb = qT_tiles[C], kT_tiles[C]
                v9t = v9_tiles.pop(t)
                # all-9-shift products per 2-head block, then ones-block
                # matmuls contract d with PSUM accumulation over blocks
                dps = [psd.tile([8, 3 * 128], fp32, tag=f"d{s}",
                                name=f"dps{s}_{t}") for s in range(3)]
                for b in range(4):
                    P9 = apool.tile([128, K9, 128], b16, tag="P9")
                    qsl = qTb[b][:, tt * 128:(tt + 1) * 128]
                    qin = qsl.unsqueeze(1).broadcast_to((128, K9, 128))
                    kbase = kTb[b][:, 0:1]
                    kin = bass.AP(
                        tensor=kbase.tensor, offset=kbase.offset + tt * 128,
                        ap=[list(kbase.ap[0]), [64, 3], [1, 3], [1, 128]])
                    nc.vector.tensor_mul(
                        P9.rearrange("p (a c) x -> p a c x", a=3), qin, kin)
                    for s in range(3):
                        nc.tensor.matmul(
                            dps[s], lhsT=onesb[b],
                            rhs=P9[:, 3 * s:3 * (s + 1), :].rearrange(
                                "p a x -> p (a x)"),
                            start=(b == 0), stop=(b == 3))
                # dots -> SBUF bf16, tiny PE transposes to token-major
                dsb = apool.tile([8, K9 * 128], b16, tag="dsb")
                for s in range(3):
                    nc.scalar.activation(dsb[:, s * 384:(s + 1) * 384], dps[s],
                                         Copy)
                dtok = psdt.tile([128, K9 * 8], b16, tag="dtok")
                dsb3 = dsb.rearrange("p (k x) -> p k x", k=K9)
                for kk in range(K9):
                    nc.tensor.transpose(dtok[:, ts(kk, 8)], dsb3[:, kk, :],
                                        ident[0:8, 0:8])
                # token-major softmax, [kk, h] axis order
                E = apool.tile([128, K9 * HEADS], fp32, tag="E")
                nc.scalar.activation(E, dtok, mybir.ActivationFunctionType.Exp)
                A = apool.tile([128, K9, HEADS], fp32, tag="A")
                nc.vector.tensor_mul(A.rearrange("p k h -> p (k h)"), E, wm_sb)
                Z = apool.tile([128, HEADS], fp32, tag="Z")
                nc.vector.tensor_reduce(
                    Z, A.rearrange("p k h -> p h k"),
                    axis=mybir.AxisListType.X, op=mybir.AluOpType.add)
                nc.vector.tensor_scalar_add(Z, Z, nw_sb)
                Zr = apool.tile([128, HEADS], fp32, tag="Zr")
                nc.vector.reciprocal(Zr, Z)
                Ab = apool.tile([128, K9, HEADS], b16, tag="Ab")
                nc.vector.tensor_mul(
                    Ab, A, Zr.unsqueeze(1).broadcast_to((128, K9, HEADS)))

                av = apool.tile([128, DIM], b16, tag="av")
                Pv = apool.tile([128, DIM], b16, tag="Pv")
                for kk in range(K9):
                    di, dj = kk // 3, kk % 3
                    AX = apool.tile([128, HEADS, HEAD_DIM], b16, tag="AX")
                    nc.scalar.activation(
                        AX, Ab[:, kk, :].unsqueeze(2).broadcast_to(
                            (128, HEADS, HEAD_DIM)), Copy)
                    AXf = AX.rearrange("p h d -> p (h d)")
                    if kk == 0:
                        nc.vector.tensor_mul(av, AXf, v9t[:, di, dj, :])
                    else:
                        nc.vector.tensor_mul(Pv, AXf, v9t[:, di, dj, :])
                        nc.vector.tensor_add(av, av, Pv)

                tp = pstp.tile([128, DIM], b16, tag="tp")
                for c in range(4):
                    nc.tensor.transpose(tp[:, ts(c, 128)], av[:, ts(c, 128)],
                                        ident)
                lhsT = opool.tile([128, DIM], b16, tag="lhsT")
                nc.scalar.activation(lhsT, tp, Copy)
                ps2 = psm2.tile([128, DIM], fp32, tag="mm2")
                for c in range(4):
                    nc.tensor.matmul(ps2, lhsT=lhsT[:, ts(c, 128)], rhs=wo_sb[c],
                                     start=(c == 0), stop=(c == 3))
                ot = opool.tile([128, DIM], fp32, tag="ot")
                nc.vector.tensor_add(ot, ps2, bbc_sb)
                nc.gpsimd.dma_start(out=out[ts(t, 128), :], in_=ot)

            # chunk-level software pipeline: mm1(C) runs one chunk ahead of
            # attention (kT right-halo of C-1 is filled during mm1(C))
            for C in range(NC4 + 1):
                if C < NC4:
                    mm1(C)
                if C >= 1:
                    for tt in range(4):
                        prefetch_v(4 * (C - 1) + tt)
                    for tt in range(4):
                        attn(4 * (C - 1) + tt)

    nc.compile()
    return nc


def _wrap_mask():
    # wmask[p, kk, h] = 0 where the dj=+-1 neighbor wraps around a grid row
    m = np.ones((128, K9, HEADS), dtype=np.float32)
    for p in range(128):
        j = p % GRID
        for kk in range(K9):
            dj = kk % 3 - 1
            if (j == 0 and dj == -1) or (j == GRID - 1 and dj == 1):
                m[p, kk, :] = 0.0
    return m.reshape(128, K9 * HEADS)


def kernel(x, w_qkv, w_out, b_out, h_img=64, w_img=64):
    from concourse.bass_utils import run_bass_kernel_spmd

    assert int(h_img) == GRID and int(w_img) == GRID
    if "nc" not in _CACHE:
        _CACHE["nc"] = _build()
    nc = _CACHE["nc"]

    wqh = np.ascontiguousarray(w_qkv.astype(bf16))
    woh = np.ascontiguousarray(w_out.astype(bf16))
    bbc = np.broadcast_to(b_out.astype(np.float32), (128, DIM)).copy()
    wm = _wrap_mask()
    # 3 wrapped window entries (one per di) at each grid-row edge
    nw = np.zeros((128, 1), dtype=np.float32)
    nw[np.arange(128) % GRID == 0] = 3.0
    nw[np.arange(128) % GRID == GRID - 1] = 3.0
    in_maps = []
    for i in range(B):
        xTi = np.ascontiguousarray(x[i].T.astype(bf16))
        in_maps.append(dict(xT=xTi, wq=wqh, wo=woh, bbc=bbc, wmask=wm, nw=nw))

    kw = {}
    if TRACE:
        kw = dict(trace=True, tmpdir=TRACE_DIR)
    res = run_bass_kernel_spmd(nc, in_maps, core_ids=list(range(B)), **kw)
    global LAST_RESULTS
    LAST_RESULTS = res
    return np.stack([r["out"] for r in res.results], axis=0)

